# revision 16
# baseline (speedup 1.0000x reference)
"""Multi-head attention (B=2, S=2048, D=2048, H=16, RoPE, causal) on 8 TRN2 cores.

Sharding: tensor-parallel over heads (2 heads/core) x batch as data.  Each core:
  phase 1: qkv projection for its 2 heads (both batches), RoPE fused into drain.
           qT,kT produced transposed [Dh, S]; v produced natural [S, Dh].
  phase 2: causal attention per (b,h) pair: s^T = kT.T @ qT blocks -> exp ->
           mask -> oT += v.T @ pT, row-sums l += ones.T @ pT (PSUM accum).
  phase 3: partial out-proj: out_partial = sum_h diag(1/l_h) oT_h.T @ Wout_h,
           with the 1/l normalization folded into the PSUM drain scale.
Host sums the 8 partial outputs and adds b_out.
"""

import numpy as np
import ml_dtypes

B, S, D = 2, 2048, 2048
H, DH = 16, 128
NCORES = 8
HPC = H // NCORES          # heads per core
T = B * S                  # 4096 tokens
SCALE = 1.0 / float(np.sqrt(DH))
ROPE_BASE = 10000.0

TC_N = T // 512            # 8 token chunks of 512 (phase 1)
KT_N = D // 128            # 16 contraction tiles
JB_N = S // 128            # 16 key blocks per sequence
IC_N = S // 512            # 4 query chunks per sequence
TT_N = S // 128            # 16 token tiles per batch (phase 3)
NC_N = D // 512            # 4 out-column chunks

_CACHE = {}


def _build_program():
    import concourse.bacc as bacc
    import concourse.mybir as mybir
    import concourse.tile as tile
    import concourse.bass as bass

    f32 = mybir.dt.float32
    bf16 = mybir.dt.bfloat16
    add = mybir.AluOpType.add
    mult = mybir.AluOpType.mult
    Exp = mybir.ActivationFunctionType.Exp
    Copy = mybir.ActivationFunctionType.Copy
    Ident = mybir.ActivationFunctionType.Identity
    PSUM = bass.MemorySpace.PSUM

    nc = bacc.Bacc("TRN2", target_bir_lowering=False, debug=False)

    xT = nc.dram_tensor("xT", [D, T], bf16, kind="ExternalInput")
    wq = nc.dram_tensor("wq", [D, HPC * DH], bf16, kind="ExternalInput")
    wk = nc.dram_tensor("wk", [D, HPC * DH], bf16, kind="ExternalInput")
    wv = nc.dram_tensor("wv", [D, HPC * DH], bf16, kind="ExternalInput")
    wo = nc.dram_tensor("wo", [HPC * DH, D], bf16, kind="ExternalInput")
    bq = nc.dram_tensor("bq", [DH, HPC], f32, kind="ExternalInput")
    bk = nc.dram_tensor("bk", [DH, HPC], f32, kind="ExternalInput")
    bvb = nc.dram_tensor("bvb", [128, HPC * DH], f32, kind="ExternalInput")
    cos2 = nc.dram_tensor("cos2", [DH, S], bf16, kind="ExternalInput")
    sin2 = nc.dram_tensor("sin2", [DH, S], bf16, kind="ExternalInput")
    masks = nc.dram_tensor("masks", [DH, 4 * 512], bf16, kind="ExternalInput")
    out = nc.dram_tensor("out", [T, D], bf16, kind="ExternalOutput")

    with tile.TileContext(nc) as tc:
        with tc.tile_pool(name="persist", bufs=1) as pp:
            # --- resident weights/constants ---
            wq_sb = pp.tile([128, KT_N * 256], bf16, tag="wq_sb", name="wq_sb")
            wk_sb = pp.tile([128, KT_N * 256], bf16, tag="wk_sb", name="wk_sb")
            wv_sb = pp.tile([128, KT_N * 256], bf16, tag="wv_sb", name="wv_sb")
            wo_sb = pp.tile([128, HPC * D], bf16, tag="wo_sb", name="wo_sb")
            cos2_sb = pp.tile([DH, S], bf16, tag="cos2_sb", name="cos2_sb")
            sin2_sb = pp.tile([DH, S], bf16, tag="sin2_sb", name="sin2_sb")
            masks_sb = pp.tile([DH, 4 * 512], bf16, tag="masks_sb", name="masks_sb")
            bq_sb = pp.tile([DH, HPC], f32, tag="bq_sb", name="bq_sb")
            bk_sb = pp.tile([DH, HPC], f32, tag="bk_sb", name="bk_sb")
            bvb_sb = pp.tile([128, HPC * DH], f32, tag="bvb_sb", name="bvb_sb")
            # all-ones stationary: ones128.T @ pt replicates colsums to all
            # 128 PSUM partitions -> denominator tile needs no broadcast
            ones_sb = pp.tile([128, 128], bf16, tag="ones_sb", name="ones_sb")
            nc.vector.memset(ones_sb[:], 1.0)

            # --- per-(b,h) persistent tensors ---
            qT, kT, vN, oT = {}, {}, {}, {}
            for b in range(B):
                for h in range(HPC):
                    qT[b, h] = pp.tile([128, S], bf16, tag=f"qT{b}{h}", name=f"qT{b}{h}")
                    kT[b, h] = pp.tile([128, S], bf16, tag=f"kT{b}{h}", name=f"kT{b}{h}")
                    vN[b, h] = pp.tile([128, S], bf16, tag=f"vN{b}{h}", name=f"vN{b}{h}")
                    oT[b, h] = pp.tile([128, S], bf16, tag=f"oT{b}{h}", name=f"oT{b}{h}")

            # ================= phase 1: qkv projection =================
            with tc.tile_pool(name="xtp", bufs=3) as xtp, \
                 tc.tile_pool(name="ps_qk", bufs=5, space=PSUM) as ps_qk, \
                 tc.tile_pool(name="ps_v", bufs=3, space=PSUM) as ps_v, \
                 tc.tile_pool(name="rtp", bufs=4) as rtp:
                # first token chunk's x DMAs go out FIRST so the PE can start
                # ~2MB in; weight/table DMAs follow and overlap compute.
                xt0 = xtp.tile([128, KT_N * 512], bf16, tag="xt", name="xt0")
                for k in range(KT_N):
                    nc.sync.dma_start(
                        xt0[:, k * 512:(k + 1) * 512], xT[k * 128:(k + 1) * 128, 0:512])
                    nc.sync.dma_start(
                        wq_sb[:, k * 256:(k + 1) * 256], wq[k * 128:(k + 1) * 128, :])
                    nc.sync.dma_start(
                        wk_sb[:, k * 256:(k + 1) * 256], wk[k * 128:(k + 1) * 128, :])
                    nc.sync.dma_start(
                        wv_sb[:, k * 256:(k + 1) * 256], wv[k * 128:(k + 1) * 128, :])
                nc.sync.dma_start(cos2_sb[:], cos2[:])
                nc.sync.dma_start(sin2_sb[:], sin2[:])
                nc.sync.dma_start(bq_sb[:], bq[:])
                nc.sync.dma_start(bk_sb[:], bk[:])
                nc.sync.dma_start(bvb_sb[:], bvb[:])
                for tcn in range(TC_N):
                    b = tcn // 4
                    s0 = (tcn % 4) * 512
                    if tcn == 0:
                        xt = xt0
                    else:
                        xt = xtp.tile([128, KT_N * 512], bf16, tag="xt", name=f"xt{tcn}")
                        for k in range(KT_N):
                            nc.sync.dma_start(
                                xt[:, k * 512:(k + 1) * 512],
                                xT[k * 128:(k + 1) * 128, tcn * 512:(tcn + 1) * 512])
                    # all 8 accumulation chains (4 q/k + 4 v) run k-major so
                    # each short v-LDWEIGHTS hides under a longer q/k stream
                    qk_tiles = []
                    for gi, (wsb, bias, dst) in enumerate(
                            ((wq_sb, bq_sb, qT), (wk_sb, bk_sb, kT))):
                        for h in range(HPC):
                            ps = ps_qk.tile([128, 512], f32, tag="psqk",
                                            name=f"psqk{tcn}{gi}{h}")
                            qk_tiles.append((ps, wsb, bias, dst, h))
                    pv = [ps_v.tile([128, 512], f32, tag="psv", name=f"psv{tcn}{hf}")
                          for hf in range(2)]
                    for k in range(KT_N):
                        for ps, wsb, bias, dst, h in qk_tiles:
                            nc.tensor.matmul(
                                ps[:],
                                wsb[:, k * 256 + h * 128: k * 256 + (h + 1) * 128],
                                xt[:, k * 512:(k + 1) * 512],
                                start=(k == 0), stop=(k == KT_N - 1))
                        for hf in range(2):
                            for sub in range(2):
                                t_sub = hf * 2 + sub
                                nc.tensor.matmul(
                                    pv[hf][:, sub * 256:(sub + 1) * 256],
                                    xt[:, k * 512 + t_sub * 128: k * 512 + (t_sub + 1) * 128],
                                    wv_sb[:, k * 256:(k + 1) * 256],
                                    start=(k == 0 and sub == 0),
                                    stop=(k == KT_N - 1 and sub == 1),
                                    skip_group_check=True)
                    for ps, wsb, bias, dst, h in qk_tiles:
                            qsb = rtp.tile([128, 512], bf16, tag="qsb", name=f"qsb{tcn}{h}{id(dst)%97}")
                            nc.scalar.activation(qsb[:], ps[:], Ident, bias=bias[:, h:h + 1])
                            # half-swapped copy (rotate_half) via SBUF->SBUF DMA:
                            # DVE ops can't cross partition boundaries.
                            qsw = rtp.tile([128, 512], bf16, tag="qsw", name=f"qsw{tcn}{h}")
                            nc.gpsimd.dma_start(qsw[0:64, :], qsb[64:128, :])
                            nc.gpsimd.dma_start(qsw[64:128, :], qsb[0:64, :])
                            t1 = rtp.tile([128, 512], bf16, tag="t1", name=f"t1_{tcn}{h}")
                            t2 = rtp.tile([128, 512], bf16, tag="t2", name=f"t2_{tcn}{h}")
                            nc.vector.tensor_tensor(
                                t1[:], qsb[:], cos2_sb[:, s0:s0 + 512], op=mult)
                            nc.vector.tensor_tensor(
                                t2[:], qsw[:], sin2_sb[:, s0:s0 + 512], op=mult)
                            nc.vector.tensor_tensor(
                                dst[b, h][:, s0:s0 + 512], t1[:], t2[:], op=add)
                    # v drains: psum halves -> per-(b,h) tiles + bias
                    for hf in range(2):
                        for sub in range(2):
                            t_sub = hf * 2 + sub
                            jblk = (tcn % 4) * 4 + t_sub
                            for h in range(HPC):
                                nc.vector.tensor_tensor(
                                    vN[b, h][:, jblk * 128:(jblk + 1) * 128],
                                    pv[hf][:, sub * 256 + h * 128: sub * 256 + (h + 1) * 128],
                                    bvb_sb[:, h * 128:(h + 1) * 128], op=add)

            # ================= phase 2 + 3, batch-interleaved =================
            # All pools open together (8 PSUM banks total) so batch 0's output
            # projection + 17MB DMA-out overlap batch 1's attention compute.
            nc.sync.dma_start(masks_sb[:], masks[:])
            for h in range(HPC):
                nc.sync.dma_start(wo_sb[:, h * D:(h + 1) * D], wo[h * 128:(h + 1) * 128, :])
            with tc.tile_pool(name="ps_s", bufs=2, space=PSUM) as ps_s, \
                 tc.tile_pool(name="ps_o", bufs=2, space=PSUM) as ps_o, \
                 tc.tile_pool(name="ps_l", bufs=2, space=PSUM) as ps_l, \
                 tc.tile_pool(name="ps3", bufs=2, space=PSUM) as ps3, \
                 tc.tile_pool(name="ptp", bufs=4) as ptp, \
                 tc.tile_pool(name="rrp", bufs=2) as rrp, \
                 tc.tile_pool(name="outp", bufs=6) as outp:
                for b in range(B):
                    # ---- attention for both heads of this batch ----
                    for h in range(HPC):
                        for ic in range(IC_N):
                            njb = ic * 4 + 4
                            pso = ps_o.tile([128, 512], f32, tag="pso", name=f"pso{b}{h}{ic}")
                            psl = ps_l.tile([128, 512], f32, tag="psl", name=f"psl{b}{h}{ic}")
                            for jb in range(njb):
                                pss = ps_s.tile([128, 512], f32, tag="pss",
                                                name=f"pss{b}{h}{ic}{jb}")
                                nc.tensor.matmul(
                                    pss[:],
                                    kT[b, h][:, jb * 128:(jb + 1) * 128],
                                    qT[b, h][:, ic * 512:(ic + 1) * 512],
                                    start=True, stop=True)
                                pt = ptp.tile([128, 512], bf16, tag="pt",
                                              name=f"pt{b}{h}{ic}{jb}")
                                nc.scalar.activation(pt[:], pss[:], Exp, scale=SCALE)
                                if jb >= ic * 4:
                                    di = jb - ic * 4
                                    nc.vector.tensor_tensor(
                                        pt[:], pt[:],
                                        masks_sb[:, di * 512:(di + 1) * 512], op=mult)
                                nc.tensor.matmul(
                                    pso[:], vN[b, h][:, jb * 128:(jb + 1) * 128], pt[:],
                                    start=(jb == 0), stop=(jb == njb - 1))
                                nc.tensor.matmul(
                                    psl[:], ones_sb[:], pt[:],
                                    start=(jb == 0), stop=(jb == njb - 1))
                            # normalize during drain: oT = pso * (1/l)
                            rr = rrp.tile([128, 512], f32, tag="rr", name=f"rr{b}{h}{ic}")
                            nc.vector.reciprocal_approx_fast(rr[:], psl[:])
                            nc.vector.tensor_tensor(
                                oT[b, h][:, ic * 512:(ic + 1) * 512], pso[:], rr[:], op=mult)
                    # ---- output projection partials for this batch ----
                    for tt in range(TT_N):
                        for ncx in range(NC_N):
                            ps = ps3.tile([128, 512], f32, tag="ps3", name=f"ps3{b}{tt}{ncx}")
                            nc.tensor.matmul(
                                ps[:],
                                oT[b, 0][:, tt * 128:(tt + 1) * 128],
                                wo_sb[:, 0 * D + ncx * 512: 0 * D + (ncx + 1) * 512],
                                start=True, stop=False)
                            nc.tensor.matmul(
                                ps[:],
                                oT[b, 1][:, tt * 128:(tt + 1) * 128],
                                wo_sb[:, 1 * D + ncx * 512: 1 * D + (ncx + 1) * 512],
                                start=False, stop=True)
                            osb = outp.tile([128, 512], bf16, tag="osb", name=f"osb{b}{tt}{ncx}")
                            if ncx % 2 == 0:
                                nc.scalar.activation(osb[:], ps[:], Copy)
                            else:
                                nc.vector.tensor_copy(osb[:], ps[:])
                            row0 = b * S + tt * 128
                            nc.sync.dma_start(
                                out[row0:row0 + 128, ncx * 512:(ncx + 1) * 512], osb[:])

    nc.compile()
    return nc


def _host_prep(x, w_qkv, b_qkv, w_out, b_out):
    """Build the 8 per-core input maps."""
    bf = ml_dtypes.bfloat16
    xT = np.ascontiguousarray(x.reshape(T, D).T).astype(bf)

    # RoPE tables: cos/sin [S, DH//2] -> stacked transposed [DH, S]
    inv_freq = 1.0 / (ROPE_BASE ** (np.arange(0, DH, 2, dtype=np.float32) / DH))
    t = np.arange(S, dtype=np.float32)
    freqs = np.outer(t, inv_freq)                       # [S, 64]
    cosT = np.cos(freqs).T.astype(np.float32)           # [64, S]
    sinT = np.sin(freqs).T.astype(np.float32)
    cos2 = np.concatenate([cosT, cosT], axis=0).astype(bf)      # [128, S]
    sin2 = np.concatenate([-sinT, sinT], axis=0).astype(bf)     # [128, S]

    # diagonal causal masks for delta in {0,128,256,384}
    jj = np.arange(128)[:, None]
    ii = np.arange(512)[None, :]
    mlist = [(jj + d <= ii).astype(np.float32) for d in (0, 128, 256, 384)]
    masks = np.concatenate(mlist, axis=1).astype(bf)            # [128, 2048]

    in_maps = []
    for c in range(NCORES):
        h0 = c * HPC
        cols = slice(h0 * DH, (h0 + HPC) * DH)
        wq_c = w_qkv[:, cols].astype(bf)
        wk_c = w_qkv[:, D + h0 * DH: D + (h0 + HPC) * DH].astype(bf)
        wv_c = w_qkv[:, 2 * D + h0 * DH: 2 * D + (h0 + HPC) * DH].astype(bf)
        wo_c = w_out[cols, :].astype(bf)
        bq_c = b_qkv[cols].reshape(HPC, DH).T.astype(np.float32)          # [128, 2]
        bk_c = b_qkv[D + h0 * DH: D + (h0 + HPC) * DH].reshape(HPC, DH).T.astype(np.float32)
        bv_c = b_qkv[2 * D + h0 * DH: 2 * D + (h0 + HPC) * DH].astype(np.float32)
        bvb_c = np.broadcast_to(bv_c[None, :], (128, HPC * DH)).copy()
        in_maps.append({
            "xT": xT, "wq": np.ascontiguousarray(wq_c), "wk": np.ascontiguousarray(wk_c),
            "wv": np.ascontiguousarray(wv_c), "wo": np.ascontiguousarray(wo_c),
            "bq": np.ascontiguousarray(bq_c), "bk": np.ascontiguousarray(bk_c),
            "bvb": bvb_c, "cos2": cos2, "sin2": sin2, "masks": masks,
        })
    return in_maps


def _get_program():
    if "nc" not in _CACHE:
        _CACHE["nc"] = _build_program()
    return _CACHE["nc"]


def run_on_hw(in_maps, trace=False, **kw):
    from concourse.bass_utils import run_bass_kernel_spmd
    nc = _get_program()
    return run_bass_kernel_spmd(nc, in_maps, core_ids=list(range(NCORES)),
                                trace=trace, **kw)


def kernel(x, w_qkv, b_qkv, w_out, b_out):
    x = np.asarray(x, dtype=np.float32)
    w_qkv = np.asarray(w_qkv, dtype=np.float32)
    b_qkv = np.asarray(b_qkv, dtype=np.float32)
    w_out = np.asarray(w_out, dtype=np.float32)
    b_out = np.asarray(b_out, dtype=np.float32)

    in_maps = _host_prep(x, w_qkv, b_qkv, w_out, b_out)
    res = run_on_hw(in_maps)
    acc = np.zeros((T, D), dtype=np.float32)
    for c in range(NCORES):
        acc += res.results[c]["out"].astype(np.float32)
    acc += b_out[None, :]
    return acc.reshape(B, S, D)


# revision 17
# speedup vs baseline: 1.0210x; 1.0210x over previous
"""Multi-head attention (B=2, S=2048, D=2048, H=16, RoPE, causal) on 8 TRN2 cores.

Sharding: tensor-parallel over heads (2 heads/core) x batch as data.  Each core:
  phase 1: qkv projection for its 2 heads (both batches), RoPE fused into drain.
           qT,kT produced transposed [Dh, S]; v produced natural [S, Dh].
  phase 2: causal attention per (b,h) pair: s^T = kT.T @ qT blocks -> exp ->
           mask -> oT += v.T @ pT, row-sums l += ones.T @ pT (PSUM accum).
  phase 3: partial out-proj: out_partial = sum_h diag(1/l_h) oT_h.T @ Wout_h,
           with the 1/l normalization folded into the PSUM drain scale.
Host sums the 8 partial outputs and adds b_out.
"""

import numpy as np
import ml_dtypes

B, S, D = 2, 2048, 2048
H, DH = 16, 128
NCORES = 8
HPC = H // NCORES          # heads per core
T = B * S                  # 4096 tokens
SCALE = 1.0 / float(np.sqrt(DH))
ROPE_BASE = 10000.0

TC_N = T // 512            # 8 token chunks of 512 (phase 1)
KT_N = D // 128            # 16 contraction tiles
JB_N = S // 128            # 16 key blocks per sequence
IC_N = S // 512            # 4 query chunks per sequence
TT_N = S // 128            # 16 token tiles per batch (phase 3)
NC_N = D // 512            # 4 out-column chunks

_CACHE = {}


def _build_program():
    import concourse.bacc as bacc
    import concourse.mybir as mybir
    import concourse.tile as tile
    import concourse.bass as bass

    f32 = mybir.dt.float32
    bf16 = mybir.dt.bfloat16
    add = mybir.AluOpType.add
    mult = mybir.AluOpType.mult
    Exp = mybir.ActivationFunctionType.Exp
    Copy = mybir.ActivationFunctionType.Copy
    Ident = mybir.ActivationFunctionType.Identity
    PSUM = bass.MemorySpace.PSUM

    nc = bacc.Bacc("TRN2", target_bir_lowering=False, debug=False)

    xT = nc.dram_tensor("xT", [D, T], bf16, kind="ExternalInput")
    wq = nc.dram_tensor("wq", [D, HPC * DH], bf16, kind="ExternalInput")
    wk = nc.dram_tensor("wk", [D, HPC * DH], bf16, kind="ExternalInput")
    wv = nc.dram_tensor("wv", [D, HPC * DH], bf16, kind="ExternalInput")
    wo = nc.dram_tensor("wo", [HPC * DH, D], bf16, kind="ExternalInput")
    bq = nc.dram_tensor("bq", [DH, HPC], f32, kind="ExternalInput")
    bk = nc.dram_tensor("bk", [DH, HPC], f32, kind="ExternalInput")
    bvb = nc.dram_tensor("bvb", [128, HPC * DH], f32, kind="ExternalInput")
    cos2 = nc.dram_tensor("cos2", [DH, S], bf16, kind="ExternalInput")
    sin2 = nc.dram_tensor("sin2", [DH, S], bf16, kind="ExternalInput")
    masks = nc.dram_tensor("masks", [DH, 4 * 512], bf16, kind="ExternalInput")
    out = nc.dram_tensor("out", [T, D], bf16, kind="ExternalOutput")

    with tile.TileContext(nc) as tc:
        with tc.tile_pool(name="persist", bufs=1) as pp:
            # --- resident weights/constants ---
            wq_sb = pp.tile([128, KT_N * 256], bf16, tag="wq_sb", name="wq_sb")
            wk_sb = pp.tile([128, KT_N * 256], bf16, tag="wk_sb", name="wk_sb")
            wv_sb = pp.tile([128, KT_N * 256], bf16, tag="wv_sb", name="wv_sb")
            wo_sb = pp.tile([128, HPC * D], bf16, tag="wo_sb", name="wo_sb")
            cos2_sb = pp.tile([DH, S], bf16, tag="cos2_sb", name="cos2_sb")
            sin2_sb = pp.tile([DH, S], bf16, tag="sin2_sb", name="sin2_sb")
            masks_sb = pp.tile([DH, 4 * 512], bf16, tag="masks_sb", name="masks_sb")
            bq_sb = pp.tile([DH, HPC], f32, tag="bq_sb", name="bq_sb")
            bk_sb = pp.tile([DH, HPC], f32, tag="bk_sb", name="bk_sb")
            bvb_sb = pp.tile([128, HPC * DH], f32, tag="bvb_sb", name="bvb_sb")
            # all-ones stationary: ones128.T @ pt replicates colsums to all
            # 128 PSUM partitions -> denominator tile needs no broadcast
            ones_sb = pp.tile([128, 128], bf16, tag="ones_sb", name="ones_sb")
            nc.vector.memset(ones_sb[:], 1.0)

            # --- per-(b,h) persistent tensors ---
            qT, kT, vN, oT = {}, {}, {}, {}
            for b in range(B):
                for h in range(HPC):
                    qT[b, h] = pp.tile([128, S], bf16, tag=f"qT{b}{h}", name=f"qT{b}{h}")
                    kT[b, h] = pp.tile([128, S], bf16, tag=f"kT{b}{h}", name=f"kT{b}{h}")
                    vN[b, h] = pp.tile([128, S], bf16, tag=f"vN{b}{h}", name=f"vN{b}{h}")
                    oT[b, h] = pp.tile([128, S], bf16, tag=f"oT{b}{h}", name=f"oT{b}{h}")

            # ================= phase 1: qkv projection =================
            with tc.tile_pool(name="xtp", bufs=3) as xtp, \
                 tc.tile_pool(name="ps_qk", bufs=5, space=PSUM) as ps_qk, \
                 tc.tile_pool(name="ps_v", bufs=3, space=PSUM) as ps_v, \
                 tc.tile_pool(name="rtp", bufs=4) as rtp:
                # first token chunk's x DMAs go out FIRST so the PE can start
                # ~2MB in; weight/table DMAs follow and overlap compute.
                xt0 = xtp.tile([128, KT_N * 512], bf16, tag="xt", name="xt0")
                for k in range(KT_N):
                    nc.sync.dma_start(
                        xt0[:, k * 512:(k + 1) * 512], xT[k * 128:(k + 1) * 128, 0:512])
                    nc.sync.dma_start(
                        wq_sb[:, k * 256:(k + 1) * 256], wq[k * 128:(k + 1) * 128, :])
                    nc.sync.dma_start(
                        wk_sb[:, k * 256:(k + 1) * 256], wk[k * 128:(k + 1) * 128, :])
                    nc.sync.dma_start(
                        wv_sb[:, k * 256:(k + 1) * 256], wv[k * 128:(k + 1) * 128, :])
                nc.sync.dma_start(cos2_sb[:], cos2[:])
                nc.sync.dma_start(sin2_sb[:], sin2[:])
                nc.sync.dma_start(bq_sb[:], bq[:])
                nc.sync.dma_start(bk_sb[:], bk[:])
                nc.sync.dma_start(bvb_sb[:], bvb[:])
                for tcn in range(TC_N):
                    b = tcn // 4
                    s0 = (tcn % 4) * 512
                    if tcn == 0:
                        xt = xt0
                    else:
                        xt = xtp.tile([128, KT_N * 512], bf16, tag="xt", name=f"xt{tcn}")
                        for k in range(KT_N):
                            nc.sync.dma_start(
                                xt[:, k * 512:(k + 1) * 512],
                                xT[k * 128:(k + 1) * 128, tcn * 512:(tcn + 1) * 512])
                    # all 8 accumulation chains (4 q/k + 4 v) run k-major so
                    # each short v-LDWEIGHTS hides under a longer q/k stream
                    qk_tiles = []
                    for gi, (wsb, bias, dst) in enumerate(
                            ((wq_sb, bq_sb, qT), (wk_sb, bk_sb, kT))):
                        for h in range(HPC):
                            ps = ps_qk.tile([128, 512], f32, tag="psqk",
                                            name=f"psqk{tcn}{gi}{h}")
                            qk_tiles.append((ps, wsb, bias, dst, h))
                    pv = [ps_v.tile([128, 512], f32, tag="psv", name=f"psv{tcn}{hf}")
                          for hf in range(2)]
                    for k in range(KT_N):
                        for ps, wsb, bias, dst, h in qk_tiles:
                            nc.tensor.matmul(
                                ps[:],
                                wsb[:, k * 256 + h * 128: k * 256 + (h + 1) * 128],
                                xt[:, k * 512:(k + 1) * 512],
                                start=(k == 0), stop=(k == KT_N - 1))
                        for hf in range(2):
                            for sub in range(2):
                                t_sub = hf * 2 + sub
                                nc.tensor.matmul(
                                    pv[hf][:, sub * 256:(sub + 1) * 256],
                                    xt[:, k * 512 + t_sub * 128: k * 512 + (t_sub + 1) * 128],
                                    wv_sb[:, k * 256:(k + 1) * 256],
                                    start=(k == 0 and sub == 0),
                                    stop=(k == KT_N - 1 and sub == 1),
                                    skip_group_check=True)
                    for ps, wsb, bias, dst, h in qk_tiles:
                            qsb = rtp.tile([128, 512], bf16, tag="qsb", name=f"qsb{tcn}{h}{id(dst)%97}")
                            nc.scalar.activation(qsb[:], ps[:], Ident, bias=bias[:, h:h + 1])
                            # half-swapped copy (rotate_half) via SBUF->SBUF DMA:
                            # DVE ops can't cross partition boundaries.
                            qsw = rtp.tile([128, 512], bf16, tag="qsw", name=f"qsw{tcn}{h}")
                            nc.gpsimd.dma_start(qsw[0:64, :], qsb[64:128, :])
                            nc.gpsimd.dma_start(qsw[64:128, :], qsb[0:64, :])
                            t1 = rtp.tile([128, 512], bf16, tag="t1", name=f"t1_{tcn}{h}")
                            t2 = rtp.tile([128, 512], bf16, tag="t2", name=f"t2_{tcn}{h}")
                            nc.vector.tensor_tensor(
                                t1[:], qsb[:], cos2_sb[:, s0:s0 + 512], op=mult)
                            nc.vector.tensor_tensor(
                                t2[:], qsw[:], sin2_sb[:, s0:s0 + 512], op=mult)
                            nc.vector.tensor_tensor(
                                dst[b, h][:, s0:s0 + 512], t1[:], t2[:], op=add)
                    # v drains: psum halves -> per-(b,h) tiles + bias
                    for hf in range(2):
                        for sub in range(2):
                            t_sub = hf * 2 + sub
                            jblk = (tcn % 4) * 4 + t_sub
                            for h in range(HPC):
                                nc.vector.tensor_tensor(
                                    vN[b, h][:, jblk * 128:(jblk + 1) * 128],
                                    pv[hf][:, sub * 256 + h * 128: sub * 256 + (h + 1) * 128],
                                    bvb_sb[:, h * 128:(h + 1) * 128], op=add)

            # ================= phase 2 + 3, batch-interleaved =================
            # All pools open together (8 PSUM banks total) so batch 0's output
            # projection + 17MB DMA-out overlap batch 1's attention compute.
            nc.sync.dma_start(masks_sb[:], masks[:])
            for h in range(HPC):
                nc.sync.dma_start(wo_sb[:, h * D:(h + 1) * D], wo[h * 128:(h + 1) * 128, :])
            with tc.tile_pool(name="ps_s", bufs=3, space=PSUM) as ps_s, \
                 tc.tile_pool(name="ps_o", bufs=2, space=PSUM) as ps_o, \
                 tc.tile_pool(name="ps_l", bufs=1, space=PSUM) as ps_l, \
                 tc.tile_pool(name="ps3", bufs=2, space=PSUM) as ps3, \
                 tc.tile_pool(name="ptp", bufs=6) as ptp, \
                 tc.tile_pool(name="rrp", bufs=2) as rrp, \
                 tc.tile_pool(name="outp", bufs=6) as outp:
                for b in range(B):
                    # ---- attention for both heads of this batch ----
                    for h in range(HPC):
                        for ic in range(IC_N):
                            njb = ic * 4 + 4
                            pso = ps_o.tile([128, 512], f32, tag="pso", name=f"pso{b}{h}{ic}")
                            psl = ps_l.tile([128, 512], f32, tag="psl", name=f"psl{b}{h}{ic}")
                            for jb in range(njb):
                                pss = ps_s.tile([128, 512], f32, tag="pss",
                                                name=f"pss{b}{h}{ic}{jb}")
                                nc.tensor.matmul(
                                    pss[:],
                                    kT[b, h][:, jb * 128:(jb + 1) * 128],
                                    qT[b, h][:, ic * 512:(ic + 1) * 512],
                                    start=True, stop=True)
                                pt = ptp.tile([128, 512], bf16, tag="pt",
                                              name=f"pt{b}{h}{ic}{jb}")
                                nc.scalar.activation(pt[:], pss[:], Exp, scale=SCALE)
                                if jb >= ic * 4:
                                    di = jb - ic * 4
                                    nc.vector.tensor_tensor(
                                        pt[:], pt[:],
                                        masks_sb[:, di * 512:(di + 1) * 512], op=mult)
                                nc.tensor.matmul(
                                    pso[:], vN[b, h][:, jb * 128:(jb + 1) * 128], pt[:],
                                    start=(jb == 0), stop=(jb == njb - 1))
                                nc.tensor.matmul(
                                    psl[:], ones_sb[:], pt[:],
                                    start=(jb == 0), stop=(jb == njb - 1))
                            # normalize during drain: oT = pso * (1/l)
                            rr = rrp.tile([128, 512], f32, tag="rr", name=f"rr{b}{h}{ic}")
                            nc.vector.reciprocal_approx_fast(rr[:], psl[:])
                            nc.vector.tensor_tensor(
                                oT[b, h][:, ic * 512:(ic + 1) * 512], pso[:], rr[:], op=mult)
                    # ---- output projection partials for this batch ----
                    for tt in range(TT_N):
                        for ncx in range(NC_N):
                            ps = ps3.tile([128, 512], f32, tag="ps3", name=f"ps3{b}{tt}{ncx}")
                            nc.tensor.matmul(
                                ps[:],
                                oT[b, 0][:, tt * 128:(tt + 1) * 128],
                                wo_sb[:, 0 * D + ncx * 512: 0 * D + (ncx + 1) * 512],
                                start=True, stop=False)
                            nc.tensor.matmul(
                                ps[:],
                                oT[b, 1][:, tt * 128:(tt + 1) * 128],
                                wo_sb[:, 1 * D + ncx * 512: 1 * D + (ncx + 1) * 512],
                                start=False, stop=True)
                            osb = outp.tile([128, 512], bf16, tag="osb", name=f"osb{b}{tt}{ncx}")
                            if ncx % 2 == 0:
                                nc.scalar.activation(osb[:], ps[:], Copy)
                            else:
                                nc.vector.tensor_copy(osb[:], ps[:])
                            row0 = b * S + tt * 128
                            nc.sync.dma_start(
                                out[row0:row0 + 128, ncx * 512:(ncx + 1) * 512], osb[:])

    nc.compile()
    return nc


def _host_prep(x, w_qkv, b_qkv, w_out, b_out):
    """Build the 8 per-core input maps."""
    bf = ml_dtypes.bfloat16
    xT = np.ascontiguousarray(x.reshape(T, D).T).astype(bf)

    # RoPE tables: cos/sin [S, DH//2] -> stacked transposed [DH, S]
    inv_freq = 1.0 / (ROPE_BASE ** (np.arange(0, DH, 2, dtype=np.float32) / DH))
    t = np.arange(S, dtype=np.float32)
    freqs = np.outer(t, inv_freq)                       # [S, 64]
    cosT = np.cos(freqs).T.astype(np.float32)           # [64, S]
    sinT = np.sin(freqs).T.astype(np.float32)
    cos2 = np.concatenate([cosT, cosT], axis=0).astype(bf)      # [128, S]
    sin2 = np.concatenate([-sinT, sinT], axis=0).astype(bf)     # [128, S]

    # diagonal causal masks for delta in {0,128,256,384}
    jj = np.arange(128)[:, None]
    ii = np.arange(512)[None, :]
    mlist = [(jj + d <= ii).astype(np.float32) for d in (0, 128, 256, 384)]
    masks = np.concatenate(mlist, axis=1).astype(bf)            # [128, 2048]

    in_maps = []
    for c in range(NCORES):
        h0 = c * HPC
        cols = slice(h0 * DH, (h0 + HPC) * DH)
        wq_c = w_qkv[:, cols].astype(bf)
        wk_c = w_qkv[:, D + h0 * DH: D + (h0 + HPC) * DH].astype(bf)
        wv_c = w_qkv[:, 2 * D + h0 * DH: 2 * D + (h0 + HPC) * DH].astype(bf)
        wo_c = w_out[cols, :].astype(bf)
        bq_c = b_qkv[cols].reshape(HPC, DH).T.astype(np.float32)          # [128, 2]
        bk_c = b_qkv[D + h0 * DH: D + (h0 + HPC) * DH].reshape(HPC, DH).T.astype(np.float32)
        bv_c = b_qkv[2 * D + h0 * DH: 2 * D + (h0 + HPC) * DH].astype(np.float32)
        bvb_c = np.broadcast_to(bv_c[None, :], (128, HPC * DH)).copy()
        in_maps.append({
            "xT": xT, "wq": np.ascontiguousarray(wq_c), "wk": np.ascontiguousarray(wk_c),
            "wv": np.ascontiguousarray(wv_c), "wo": np.ascontiguousarray(wo_c),
            "bq": np.ascontiguousarray(bq_c), "bk": np.ascontiguousarray(bk_c),
            "bvb": bvb_c, "cos2": cos2, "sin2": sin2, "masks": masks,
        })
    return in_maps


def _get_program():
    if "nc" not in _CACHE:
        _CACHE["nc"] = _build_program()
    return _CACHE["nc"]


def run_on_hw(in_maps, trace=False, **kw):
    from concourse.bass_utils import run_bass_kernel_spmd
    nc = _get_program()
    return run_bass_kernel_spmd(nc, in_maps, core_ids=list(range(NCORES)),
                                trace=trace, **kw)


def kernel(x, w_qkv, b_qkv, w_out, b_out):
    x = np.asarray(x, dtype=np.float32)
    w_qkv = np.asarray(w_qkv, dtype=np.float32)
    b_qkv = np.asarray(b_qkv, dtype=np.float32)
    w_out = np.asarray(w_out, dtype=np.float32)
    b_out = np.asarray(b_out, dtype=np.float32)

    in_maps = _host_prep(x, w_qkv, b_qkv, w_out, b_out)
    res = run_on_hw(in_maps)
    acc = np.zeros((T, D), dtype=np.float32)
    for c in range(NCORES):
        acc += res.results[c]["out"].astype(np.float32)
    acc += b_out[None, :]
    return acc.reshape(B, S, D)
